# revision 3
# baseline (speedup 1.0000x reference)
"""Multi-head causal attention (B=2, S=4096, D=1024, H=16) on 8 TRN2 NeuronCores.

Sharding: head-parallel. Core c computes heads 2c, 2c+1 (128 of the 1024
projection columns) for both batches:
  - QKV column-parallel: each core gets Wq/Wk/Wv[:, c*128:(c+1)*128]
  - attention for its 2 heads over all tokens (causal)
  - out-proj row-parallel: partial_out = ctx_c @ Wo[c*128:(c+1)*128, :]
  - host sums the 8 partials and adds bo.

Engine layout (chosen against the TRN2 cost model):
  - PE: QKV projections, score matmuls (keys on psum partitions), ctx
    matmuls restructured with the exp'd scores as the STATIONARY operand
    and v (+ a ones column for the softmax denominator) as the 65-wide
    MOVING operand -- this halves ctx PE-rows vs. moving-exp form.
    Plus [q,dh]->[dh,q] ctx transposes and the out-projection.
  - ACT (scalar): the exp activations (the serial co-bottleneck).
  - DVE (vector): all psum evacuations (q/k/v, ctx, out-proj), softmax
    normalize (reciprocal + per-partition tensor_scalar mult), and a
    1/6 share of the exp tiles via the Schraudolph int16 bit trick
    (bf16 bits of e^x = int16(128/ln2 * x + 16248.5), ~1.8% rms on
    those weights; end-to-end rel err 6.1e-3 vs the 2e-2 budget).
  - Pool (gpsimd): causal-mask multiplies (GPSIMD cannot touch PSUM).
Scheduling: one continuous jt stream across chunks with a lag-10 ctx
software pipeline, QKV windows emitted as self-contained filler steps
between attention ops, and deadline-free out-projection steps in a
reserve queue that feeds the ACT-bound late chunks.

Layouts on-chip (per core):
  qT, kT:  [128, T]  rows 0:64 head0, 64:128 head1 (transposed projections)
  vA:      [128, T/128, 130]  per key-tile [v_h0 | ones | v_h1 | ones]
  sc:      PSUM [128 keys, 2 heads, IC queries] -> exp on ACT -> ex SBUF
  cx:      PSUM [128 queries, 2*130] two query-tiles' ctx (+denominators
           in columns 64/129 via the ones columns)
  cT:      [128 = 2*64 dh, T]  normalized+transposed ctx
"""

from collections import deque
from contextlib import ExitStack

import numpy as np

import concourse.bass as bass
import concourse.tile as tile
from concourse import bacc, mybir

F32 = mybir.dt.float32
# 16-bit storage dtype: fp16 (10 mantissa bits) instead of bf16 -- ~8x
# lower quantization error at identical engine/matmul cost, freeing error
# budget for a much larger Schraudolph share of the exp work.
BF16 = mybir.dt.float16
P = 128
AF = mybir.ActivationFunctionType
ALU = mybir.AluOpType

N_CORES = 8
B_FULL, S_FULL, D_FULL, H_FULL = 2, 4096, 1024, 16
DH = 64
CW = 128  # projection columns per core (2 heads * 64)


def build_program(S=S_FULL, B=B_FULL, D=D_FULL):
    """Build the per-core Bass program (same program on all 8 cores)."""
    T = B * S
    KC = D // P            # contraction chunks for the projections
    IC = min(512, S)       # query-chunk width
    QT = IC // P           # 128-query tiles per chunk
    NJ = S // P            # key tiles per batch
    NIC = S // IC          # query chunks per batch
    WN = min(512, T)       # QKV token window
    nwin = T // WN

    nc = bacc.Bacc("TRN2", target_bir_lowering=False, debug=False,
                   num_devices=N_CORES)

    xT = nc.dram_tensor("xT", [D, T], BF16, kind="ExternalInput").ap()
    wq = nc.dram_tensor("wq", [P, KC, CW], BF16, kind="ExternalInput").ap()
    wk = nc.dram_tensor("wk", [P, KC, CW], BF16, kind="ExternalInput").ap()
    wv = nc.dram_tensor("wv", [P, KC, CW], BF16, kind="ExternalInput").ap()
    wo = nc.dram_tensor("wo", [CW, D], BF16, kind="ExternalInput").ap()
    mask = nc.dram_tensor("mask", [P, P], BF16, kind="ExternalInput").ap()
    ident = nc.dram_tensor("ident", [P, P], BF16, kind="ExternalInput").ap()
    out = nc.dram_tensor("out", [T, D], BF16, kind="ExternalOutput").ap()

    with tile.TileContext(nc) as tc, ExitStack() as ctx:
        singles = ctx.enter_context(tc.tile_pool(name="singles", bufs=1))
        qT = singles.tile([P, T], BF16, name="qT")
        kT = singles.tile([P, T], BF16, name="kT")
        vA = singles.tile([P, B * NJ, 130], BF16, name="vA")
        cT = singles.tile([P, T], BF16, name="cT")
        wq_s = singles.tile([P, KC, CW], BF16, name="wq_s")
        wk_s = singles.tile([P, KC, CW], BF16, name="wk_s")
        wv_s = singles.tile([P, KC, CW], BF16, name="wv_s")
        wo_s = singles.tile([CW, D], BF16, name="wo_s")
        mask_s = singles.tile([P, P], BF16, name="mask_s")
        ident_s = singles.tile([P, P], BF16, name="ident_s")

        # wq first: the very first matmuls need only wq + xw[0]; the rest
        # of the weight loads are interleaved into window 0's DMA step.
        nc.sync.dma_start(out=wq_s, in_=wq)
        nc.vector.memset(vA[:, :, 64:65], 1.0)
        nc.vector.memset(vA[:, :, 129:130], 1.0)

        xw_pool = ctx.enter_context(tc.tile_pool(name="xw_pool", bufs=4))
        # PSUM budget (8 banks): sc 2x2 + cx 2x1 + sm 2x1 = 8
        sc_ps = ctx.enter_context(
            tc.tile_pool(name="sc_ps", bufs=2, space=bass.MemorySpace.PSUM))
        cx_ps = ctx.enter_context(
            tc.tile_pool(name="cx_ps", bufs=2, space=bass.MemorySpace.PSUM))
        sm_ps = ctx.enter_context(
            tc.tile_pool(name="sm_ps", bufs=2, space=bass.MemorySpace.PSUM))
        import os as _os
        exp_sb = ctx.enter_context(tc.tile_pool(
            name="exp_sb", bufs=int(_os.environ.get('K_EXB', '24'))))
        ctxn_sb = ctx.enter_context(tc.tile_pool(
            name="ctxn_sb", bufs=int(_os.environ.get("K_CNB", "12"))))
        dn_sb = ctx.enter_context(tc.tile_pool(name="dn_sb", bufs=4))
        ob_sb = ctx.enter_context(tc.tile_pool(
            name="ob_sb", bufs=int(_os.environ.get("K_OBB", "6"))))

        fill = deque()          # deferred emission steps (mostly PE filler)
        reserve = deque()       # deadline-free steps (out-projection):
        # drained only when fill is dry, feeding the filler-starved
        # ACT-bound late chunks
        win_emitted = [False] * nwin
        win_done = [False] * nwin

        allow_rsv = {"on": True, "keep": 0}

        def pump(n):
            for _ in range(n):
                if fill:
                    fill.popleft()()
                elif (reserve and allow_rsv["on"]
                        and len(reserve) > allow_rsv["keep"]):
                    reserve.popleft()()

        def window_steps(w):
            """Generate the emission steps for QKV window w."""
            xw = xw_pool.tile([P, KC, WN], BF16, name="xw", tag="xw")

            # DMAs issue eagerly at push time (no PE work): the loads are
            # in flight well before the compute steps get popped.
            if w == 0:
                # window 0 gates kernel start: per-kc loads spread over two
                # queues let the first matmul begin as soon as its slice lands
                for kc in range(KC):
                    eng = nc.scalar if kc % 2 == 1 else nc.sync
                    eng.dma_start(
                        out=xw[:, kc, :],
                        in_=xT[kc * P:(kc + 1) * P, w * WN:(w + 1) * WN])
            else:
                # one strided DMA for the whole window: [p, kc, wn] <-
                # xT[kc*P+p, w*WN+wn] (saves 7 HWDGE passes + SP issues)
                xsrc = bass.AP(tensor=xT.tensor, offset=w * WN,
                               ap=[[T, P], [P * T, KC], [1, WN]])
                nc.sync.dma_start(out=xw, in_=xsrc)
            if w == 0:
                # deferred loads, ordered by first use
                nc.sync.dma_start(out=wk_s, in_=wk)
                nc.sync.dma_start(out=mask_s, in_=mask)
                nc.sync.dma_start(out=wv_s, in_=wv)
                nc.sync.dma_start(out=ident_s, in_=ident)
                nc.sync.dma_start(out=wo_s, in_=wo)

            state = {}
            # window 0 runs at kernel start with nothing to overlap:
            # per-kc steps let the first matmul start as soon as its own
            # wq/xw slices land. Later windows use ~850ns halves.
            nparts = KC if w == 0 else 2

            def proj_step(which, w_sb, dst, part):
                # fill holds only window steps so the parts pop adjacently
                # (no sm-pool interleave hazard)
                def step():
                    if part == 0:
                        state[which] = sm_ps.tile([P, WN], F32,
                                                  name=which, tag="sm")
                    ps = state[which]
                    for kc in range(part * KC // nparts,
                                    (part + 1) * KC // nparts):
                        nc.tensor.matmul(ps, w_sb[:, kc, :], xw[:, kc, :],
                                         start=(kc == 0),
                                         stop=(kc == KC - 1))
                    if part == nparts - 1:
                        nc.vector.tensor_copy(
                            dst[:, w * WN:(w + 1) * WN], ps)
                return step

            for part in range(nparts):
                yield proj_step("q_ps", wq_s, qT, part)
            for part in range(nparts):
                yield proj_step("k_ps", wk_s, kT, part)

            def v_step(st):
                def step():
                    jt = (w * WN) // P + st  # global token tile
                    vp = sm_ps.tile([P, CW], F32, name="vp", tag="sm")
                    for kc in range(KC):
                        nc.tensor.matmul(vp, xw[:, kc, st * P:(st + 1) * P],
                                         wv_s[:, kc, :],
                                         start=(kc == 0), stop=(kc == KC - 1))
                    # strided evac: [v_h0 | v_h1] -> vA cols {0:64, 65:129}
                    base = vA[:, jt, 0:64]
                    dst = bass.AP(tensor=base.tensor, offset=base.offset,
                                  ap=[base.ap[0], [65, 2], [1, 64]])
                    src = bass.AP(tensor=vp.tensor, offset=vp.offset,
                                  ap=[vp.ap[0], [64, 2], [1, 64]])
                    nc.vector.tensor_copy(dst, src)
                return step

            for st in range(WN // P):
                yield v_step(st)

        def push_window(w):
            if w >= nwin or win_emitted[w]:
                return
            win_emitted[w] = True
            for s in window_steps(w):
                fill.append(s)

            def marker():
                win_done[w] = True
            fill.append(marker)

        def need_now(w):
            """Window w must be fully emitted before returning.

            Drains the FIFO only up to window w's own completion marker,
            leaving later windows / tail steps queued as jt-loop filler.
            """
            w = min(w, nwin - 1)
            push_window(w)
            while not win_done[w]:
                fill.popleft()()

        def finalize_qtile(cxs, gi0, icn, qt):
            """Normalize + transpose + out-project one completed qtile.

            Emitted immediately (gates reuse of the qtile's cx psum
            region): DVE reciprocal + raw-ctx evacuation, and Pool builds
            of diag(1/denom) = ident * recip. Queued as PE filler: the
            fused normalize-transpose matmuls (ctxU^T @ diag), the cT
            evacuation, and the out-projection.
            """
            cx = cxs[qt // 2]
            s = (qt % 2) * 130
            # evacuate + normalize: reciprocal of the psum denominator
            # columns, then one mult per head with the per-partition scalar
            # (hardware tensor_scalar has no divide)
            dn = dn_sb.tile([P, 2], F32, name="dn", tag="dn")
            nc.vector.reciprocal(dn, cx[:, s + 64:s + 130:65])
            cu = ctxn_sb.tile([P, 2, 64], BF16, name="cu", tag="cn")
            base = cx[:, s:s + 64]
            vsrc = bass.AP(tensor=base.tensor, offset=base.offset,
                           ap=[base.ap[0], [65, 2], [1, 64]])
            dnb = bass.AP(tensor=dn.tensor, offset=dn.offset,
                          ap=[dn.ap[0], [1, 2], [0, 64]])
            nc.vector.tensor_mul(cu, vsrc, dnb)
            s0 = gi0 + qt * P
            state = {}

            def t_step():
                tp = sm_ps.tile([P, P], BF16, name="tp", tag="sm")
                nc.tensor.transpose(tp, cu[:, :, :], ident_s)
                nc.vector.tensor_copy(cT[:, s0:s0 + P], tp)

            def o_step(nn):
                def step():
                    if nn == 0:
                        state["ob"] = ob_sb.tile([P, D], BF16,
                                                 name="ob", tag="ob")
                    ob = state["ob"]
                    op = sm_ps.tile([P, 512], F32, name="op", tag="sm")
                    nc.tensor.matmul(op, cT[:, s0:s0 + P],
                                     wo_s[:, nn * 512:(nn + 1) * 512],
                                     start=True, stop=True)
                    if OB_ACT:
                        nc.scalar.copy(ob[:, nn * 512:(nn + 1) * 512], op)
                    else:
                        nc.vector.tensor_copy(
                            ob[:, nn * 512:(nn + 1) * 512], op)
                    if nn == D // 512 - 1:
                        if OUT_GP:
                            nc.gpsimd.dma_start(out=out[s0:s0 + P, :],
                                                in_=ob)
                        else:
                            nc.sync.dma_start(out=out[s0:s0 + P, :], in_=ob)
                return step

            return [t_step], [o_step(nn) for nn in range(D // 512)]

        # lag-N software pipeline for ctx matmuls, crossing chunk borders
        import os
        LAG = int(os.environ.get('K_LAG', '10'))
        RSV_MIN = int(os.environ.get('K_RSVMIN', '0'))
        TAILS_RSV = os.environ.get('K_TAILSRSV', '0') == '1'
        # fraction SCH_NUM/SCH_DEN of off-diagonal exp tiles computed on
        # DVE via the Schraudolph bit-trick instead of ACT
        SCH_NUM = int(os.environ.get('K_SCHNUM', '2'))
        SCH_DEN = int(os.environ.get('K_SCHDEN', '5'))
        SCH_PH = int(os.environ.get('K_SCHPH', '12'))
        sch_st = {"i": 0}
        OB_ACT = os.environ.get('K_OBACT', '1') == '1'
        OUT_GP = os.environ.get('K_OUTGP', '0') == '1'
        stash = deque()

        def emit_attn_chunk(b, icn):
            gi0 = b * S + icn * IC   # global query start
            njt = (icn + 1) * QT
            # bank tail work during PE-bound (small) chunks; spend it in
            # the ACT-bound (large) ones, and hold back a floor stock for
            # the filler-starved final chunks
            allow_rsv["on"] = njt >= RSV_MIN
            allow_rsv["keep"] = (0 if (b == B - 1 and icn >= NIC - 2)
                                 else int(os.environ.get('K_KEEP', '0')))
            ncx = (QT + 1) // 2
            cxs = [cx_ps.tile([P, 260], F32, name="cx", tag="cx")
                   for _ in range(ncx)]

            def make_ctx(jt, ex):
                def emit():
                    # One psum accumulation group per BANK (zero region):
                    # only the first matmul into each cx bank starts
                    # (lazily zeroing the whole bank); only the last one
                    # stops.
                    gjt = b * NJ + jt
                    for qt in range(QT):
                        if qt < jt - icn * QT:
                            continue  # fully masked block
                        cx = cxs[qt // 2]
                        s = (qt % 2) * 130
                        lastq = min(2 * (qt // 2) + 1, QT - 1)
                        for h in range(2):
                            nc.tensor.matmul(
                                cx[:, s + h * 65:s + (h + 1) * 65],
                                ex[:, h, qt * P:(qt + 1) * P],
                                vA[:, gjt, h * 65:(h + 1) * 65],
                                start=(jt == 0 and qt % 2 == 0 and h == 0),
                                stop=(jt == icn * QT + qt and qt == lastq
                                      and h == 1))
                    dq = jt - icn * QT
                    if dq >= 0 and (dq % 2 == 1 or dq == QT - 1):
                        # this cx bank's accumulation group just stopped:
                        # finalize both of its qtiles. All tail steps go
                        # to the paced reserve: the 1-pop-per-jt cadence
                        # spaces each PE step past its input evacuation.
                        all_t, all_o = [], []
                        for qt in range(2 * (dq // 2), dq + 1):
                            tsteps, osteps = finalize_qtile(
                                cxs, gi0, icn, qt)
                            all_t += tsteps
                            all_o += osteps
                        # both transposes first: doubles the spacing
                        # between each cT evacuation and its out-proj read
                        (reserve if TAILS_RSV else fill).extend(all_t)
                        reserve.extend(all_o)
                return emit

            for jt in range(njt):
                il0 = max(0, jt * P - icn * IC)
                gj0 = b * S + jt * P
                sc = sc_ps.tile([P, 2, IC], F32, name="sc", tag="sc")
                for h in range(2):
                    hp = h * 64
                    nc.tensor.matmul(
                        sc[:, h, il0:IC],
                        kT[hp:hp + 64, gj0:gj0 + P],
                        qT[hp:hp + 64, gi0 + il0:gi0 + IC],
                        start=True, stop=True)
                ex = exp_sb.tile([P, 2, IC], BF16, name="ex", tag="ex")
                diag = jt >= icn * QT
                sch_st["i"] += 1
                if (not diag and SCH_NUM and (sch_st["i"] + SCH_PH) % SCH_DEN < SCH_NUM):
                    # Schraudolph bit-trick exp on DVE (off-diagonal tiles
                    # only, ~1.8% rms weight error): fp16 bits of e^(s/8)
                    # are int16(round(1024/(8 ln2) * s + 15300.5)); one fused
                    # mult+add with int16 output aliasing the bf16 tile.
                    nc.vector.tensor_scalar(
                        ex[:, :, il0:IC].bitcast(mybir.dt.int16),
                        sc[:, :, il0:IC], 184.6649652337873, 15300.5,
                        ALU.mult, ALU.add)
                else:
                    nc.scalar.activation(ex[:, :, il0:IC], sc[:, :, il0:IC],
                                         AF.Exp, scale=0.125)
                if diag:  # diagonal tile: mask both heads in one Pool
                    # op (mask broadcast over the head dim; the ctx lag
                    # gives plenty of slack for Q7 latency)
                    d0 = jt * P - icn * IC
                    exd = ex[:, :, d0:d0 + P]
                    mb = bass.AP(tensor=mask_s.tensor, offset=mask_s.offset,
                                 ap=[mask_s.ap[0], [0, 2], [1, P]])
                    nc.gpsimd.tensor_mul(exd, exd, mb)
                pump(1)
                lag_now = 8 if (b == B - 1 and icn == NIC - 1) else LAG
                while len(stash) >= lag_now:
                    stash.popleft()()
                stash.append(make_ctx(jt, ex))
                pump(1)

        # -- main emission ---------------------------------------------
        push_window(0)
        need_now(0)
        push_window(1)
        import json as _json
        _ord = _json.loads(os.environ.get('K_ORD', 'null'))
        for b in range(B):
            icn_order = list(range(NIC))
            if _ord and b == B - 1:
                icn_order = [i for i in _ord if i < NIC]
                icn_order += [i for i in range(NIC) if i not in icn_order]
            for icn in icn_order:
                wlast = (b * S + (icn + 1) * IC - 1) // WN
                need_now(wlast)
                push_window(wlast + 1)
                push_window(wlast + 2)
                emit_attn_chunk(b, icn)
        while stash:
            stash.popleft()()
            pump(2)
        while fill:
            fill.popleft()()
        while reserve:
            reserve.popleft()()

    nc.compile()
    return nc


def _warrange(w, bf16):
    # [D, CW] -> [P, D//P, CW] contiguous (the SBUF layout, so the DMA is
    # a single contiguous copy instead of 256B strided pieces)
    D, CW_ = w.shape
    return np.ascontiguousarray(
        w.reshape(D // P, P, CW_).transpose(1, 0, 2)).astype(bf16)


def make_in_maps(x, Wq, Wk, Wv, Wo):
    bf16 = np.float16
    B, S, D = x.shape
    xT = np.ascontiguousarray(x.reshape(B * S, D).T).astype(bf16)
    mask = np.triu(np.ones((P, P), dtype=bf16))
    ident = np.eye(P, dtype=bf16)
    in_maps = []
    for c in range(N_CORES):
        cs = slice(c * CW, (c + 1) * CW)
        in_maps.append({
            "xT": xT,
            "wq": _warrange(Wq[:, cs], bf16),
            "wk": _warrange(Wk[:, cs], bf16),
            "wv": _warrange(Wv[:, cs], bf16),
            "wo": np.ascontiguousarray(Wo[cs, :]).astype(bf16),
            "mask": mask,
            "ident": ident,
        })
    return in_maps


_CACHED_NC = None


def kernel(x, Wq, Wk, Wv, Wo, bo, _trace=False):
    from concourse.bass_utils import run_bass_kernel_spmd
    global _CACHED_NC
    x = np.asarray(x, dtype=np.float32)
    B, S, D = x.shape
    if _CACHED_NC is None:
        _CACHED_NC = build_program(S=S, B=B, D=D)
    nc = _CACHED_NC
    in_maps = make_in_maps(x, np.asarray(Wq), np.asarray(Wk),
                           np.asarray(Wv), np.asarray(Wo))
    res = None
    for attempt in range(3):
        try:
            res = run_bass_kernel_spmd(nc, in_maps, list(range(N_CORES)),
                                       trace=_trace)
            break
        except Exception:
            if attempt == 2:
                raise
    out = np.zeros((B * S, D), dtype=np.float32)
    for c in range(N_CORES):
        out += res.results[c]["out"].astype(np.float32)
    out += np.asarray(bo, dtype=np.float32)[None, :]
    if _trace:
        kernel._last_result = res
    return out.reshape(B, S, D)



# revision 6
# speedup vs baseline: 1.0618x; 1.0618x over previous
"""Multi-head causal attention (B=2, S=4096, D=1024, H=16) on 8 TRN2 NeuronCores.

Sharding: head-parallel. Core c computes heads 2c, 2c+1 (128 of the 1024
projection columns) for both batches:
  - QKV column-parallel: each core gets Wq/Wk/Wv[:, c*128:(c+1)*128]
  - attention for its 2 heads over all tokens (causal)
  - out-proj row-parallel: partial_out = ctx_c @ Wo[c*128:(c+1)*128, :]
  - host sums the 8 partials and adds bo.

Engine layout (chosen against the TRN2 cost model):
  - PE: QKV projections, score matmuls (keys on psum partitions), ctx
    matmuls restructured with the exp'd scores as the STATIONARY operand
    and v (+ a ones column for the softmax denominator) as the 65-wide
    MOVING operand -- this halves ctx PE-rows vs. moving-exp form.
    Plus [q,dh]->[dh,q] ctx transposes and the out-projection.
  - ACT (scalar): the exp activations (the serial co-bottleneck).
  - DVE (vector): all psum evacuations (q/k/v, ctx, out-proj), softmax
    normalize (reciprocal + per-partition tensor_scalar mult), and a
    1/6 share of the exp tiles via the Schraudolph int16 bit trick
    (bf16 bits of e^x = int16(128/ln2 * x + 16248.5), ~1.8% rms on
    those weights; end-to-end rel err 6.1e-3 vs the 2e-2 budget).
  - Pool (gpsimd): causal-mask multiplies (GPSIMD cannot touch PSUM).
Scheduling: one continuous jt stream across chunks with a lag-10 ctx
software pipeline, QKV windows emitted as self-contained filler steps
between attention ops, and deadline-free out-projection steps in a
reserve queue that feeds the ACT-bound late chunks.

Layouts on-chip (per core):
  qT, kT:  [128, T]  rows 0:64 head0, 64:128 head1 (transposed projections)
  vA:      [128, T/128, 130]  per key-tile [v_h0 | ones | v_h1 | ones]
  sc:      PSUM [128 keys, 2 heads, IC queries] -> exp on ACT -> ex SBUF
  cx:      PSUM [128 queries, 2*130] two query-tiles' ctx (+denominators
           in columns 64/129 via the ones columns)
  cT:      [128 = 2*64 dh, T]  normalized+transposed ctx
"""

from collections import deque
from contextlib import ExitStack

import numpy as np

import concourse.bass as bass
import concourse.tile as tile
from concourse import bacc, mybir

F32 = mybir.dt.float32
# 16-bit storage dtype: fp16 (10 mantissa bits) instead of bf16 -- ~8x
# lower quantization error at identical engine/matmul cost, freeing error
# budget for a much larger Schraudolph share of the exp work.
BF16 = mybir.dt.float16
P = 128
AF = mybir.ActivationFunctionType
ALU = mybir.AluOpType

N_CORES = 8
B_FULL, S_FULL, D_FULL, H_FULL = 2, 4096, 1024, 16
DH = 64
CW = 128  # projection columns per core (2 heads * 64)


def build_program(S=S_FULL, B=B_FULL, D=D_FULL):
    """Build the per-core Bass program (same program on all 8 cores)."""
    T = B * S
    KC = D // P            # contraction chunks for the projections
    IC = min(512, S)       # query-chunk width
    QT = IC // P           # 128-query tiles per chunk
    NJ = S // P            # key tiles per batch
    NIC = S // IC          # query chunks per batch
    WN = min(512, T)       # QKV token window
    nwin = T // WN

    nc = bacc.Bacc("TRN2", target_bir_lowering=False, debug=False,
                   num_devices=N_CORES)

    xT = nc.dram_tensor("xT", [D, T], BF16, kind="ExternalInput").ap()
    wq = nc.dram_tensor("wq", [P, KC, CW], BF16, kind="ExternalInput").ap()
    wk = nc.dram_tensor("wk", [P, KC, CW], BF16, kind="ExternalInput").ap()
    wv = nc.dram_tensor("wv", [P, KC, CW], BF16, kind="ExternalInput").ap()
    wo = nc.dram_tensor("wo", [CW, D], BF16, kind="ExternalInput").ap()
    mask = nc.dram_tensor("mask", [P, P], BF16, kind="ExternalInput").ap()
    ident = nc.dram_tensor("ident", [P, P], BF16, kind="ExternalInput").ap()
    out = nc.dram_tensor("out", [T, D], BF16, kind="ExternalOutput").ap()

    with tile.TileContext(nc) as tc, ExitStack() as ctx:
        singles = ctx.enter_context(tc.tile_pool(name="singles", bufs=1))
        qT = singles.tile([P, T], BF16, name="qT")
        kT = singles.tile([P, T], BF16, name="kT")
        vA = singles.tile([P, B * NJ, 130], BF16, name="vA")
        cT = singles.tile([P, T], BF16, name="cT")
        wq_s = singles.tile([P, KC, CW], BF16, name="wq_s")
        wk_s = singles.tile([P, KC, CW], BF16, name="wk_s")
        wv_s = singles.tile([P, KC, CW], BF16, name="wv_s")
        wo_s = singles.tile([CW, D], BF16, name="wo_s")
        mask_s = singles.tile([P, P], BF16, name="mask_s")
        ident_s = singles.tile([P, P], BF16, name="ident_s")

        # wq first: the very first matmuls need only wq + xw[0]; the rest
        # of the weight loads are interleaved into window 0's DMA step.
        nc.sync.dma_start(out=wq_s, in_=wq)
        nc.vector.memset(vA[:, :, 64:65], 1.0)
        nc.vector.memset(vA[:, :, 129:130], 1.0)

        xw_pool = ctx.enter_context(tc.tile_pool(name="xw_pool", bufs=4))
        # PSUM budget (8 banks): sc 2x2 + cx 2x1 + sm 2x1 = 8
        sc_ps = ctx.enter_context(
            tc.tile_pool(name="sc_ps", bufs=2, space=bass.MemorySpace.PSUM))
        cx_ps = ctx.enter_context(
            tc.tile_pool(name="cx_ps", bufs=2, space=bass.MemorySpace.PSUM))
        sm_ps = ctx.enter_context(
            tc.tile_pool(name="sm_ps", bufs=2, space=bass.MemorySpace.PSUM))
        import os as _os
        exp_sb = ctx.enter_context(tc.tile_pool(
            name="exp_sb", bufs=int(_os.environ.get('K_EXB', '24'))))
        ctxn_sb = ctx.enter_context(tc.tile_pool(
            name="ctxn_sb", bufs=int(_os.environ.get("K_CNB", "12"))))
        dn_sb = ctx.enter_context(tc.tile_pool(name="dn_sb", bufs=4))
        ob_sb = ctx.enter_context(tc.tile_pool(
            name="ob_sb", bufs=int(_os.environ.get("K_OBB", "6"))))

        fill = deque()          # deferred emission steps (mostly PE filler)
        reserve = deque()       # deadline-free steps (out-projection):
        # drained only when fill is dry, feeding the filler-starved
        # ACT-bound late chunks
        win_emitted = [False] * nwin
        win_done = [False] * nwin

        allow_rsv = {"on": True, "keep": 0}

        def pump(n):
            for _ in range(n):
                if fill:
                    fill.popleft()()
                elif (reserve and allow_rsv["on"]
                        and len(reserve) > allow_rsv["keep"]):
                    reserve.popleft()()

        def window_steps(w):
            """Generate the emission steps for QKV window w."""
            xw = xw_pool.tile([P, KC, WN], BF16, name="xw", tag="xw")

            # DMAs issue eagerly at push time (no PE work): the loads are
            # in flight well before the compute steps get popped.
            if w == 0:
                # window 0 gates kernel start: per-kc loads spread over two
                # queues let the first matmul begin as soon as its slice lands
                for kc in range(KC):
                    eng = nc.scalar if kc % 2 == 1 else nc.sync
                    eng.dma_start(
                        out=xw[:, kc, :],
                        in_=xT[kc * P:(kc + 1) * P, w * WN:(w + 1) * WN])
            else:
                # one strided DMA for the whole window: [p, kc, wn] <-
                # xT[kc*P+p, w*WN+wn] (saves 7 HWDGE passes + SP issues)
                xsrc = bass.AP(tensor=xT.tensor, offset=w * WN,
                               ap=[[T, P], [P * T, KC], [1, WN]])
                nc.sync.dma_start(out=xw, in_=xsrc)
            if w == 0:
                # deferred loads, ordered by first use
                nc.sync.dma_start(out=wk_s, in_=wk)
                nc.sync.dma_start(out=mask_s, in_=mask)
                nc.sync.dma_start(out=wv_s, in_=wv)
                nc.sync.dma_start(out=ident_s, in_=ident)
                nc.sync.dma_start(out=wo_s, in_=wo)

            state = {}
            # window 0 runs at kernel start with nothing to overlap:
            # per-kc steps let the first matmul start as soon as its own
            # wq/xw slices land. Later windows use ~850ns halves.
            nparts = KC if w == 0 else 2

            def proj_step(which, w_sb, dst, part):
                # fill holds only window steps so the parts pop adjacently
                # (no sm-pool interleave hazard)
                def step():
                    if part == 0:
                        state[which] = sm_ps.tile([P, WN], F32,
                                                  name=which, tag="sm")
                    ps = state[which]
                    for kc in range(part * KC // nparts,
                                    (part + 1) * KC // nparts):
                        nc.tensor.matmul(ps, w_sb[:, kc, :], xw[:, kc, :],
                                         start=(kc == 0),
                                         stop=(kc == KC - 1))
                    if part == nparts - 1:
                        nc.vector.tensor_copy(
                            dst[:, w * WN:(w + 1) * WN], ps)
                return step

            for part in range(nparts):
                yield proj_step("q_ps", wq_s, qT, part)
            for part in range(nparts):
                yield proj_step("k_ps", wk_s, kT, part)

            def v_step(st):
                def step():
                    jt = (w * WN) // P + st  # global token tile
                    vp = sm_ps.tile([P, CW], F32, name="vp", tag="sm")
                    for kc in range(KC):
                        nc.tensor.matmul(vp, xw[:, kc, st * P:(st + 1) * P],
                                         wv_s[:, kc, :],
                                         start=(kc == 0), stop=(kc == KC - 1))
                    # strided evac: [v_h0 | v_h1] -> vA cols {0:64, 65:129}
                    base = vA[:, jt, 0:64]
                    dst = bass.AP(tensor=base.tensor, offset=base.offset,
                                  ap=[base.ap[0], [65, 2], [1, 64]])
                    src = bass.AP(tensor=vp.tensor, offset=vp.offset,
                                  ap=[vp.ap[0], [64, 2], [1, 64]])
                    nc.vector.tensor_copy(dst, src)
                return step

            for st in range(WN // P):
                yield v_step(st)

        def push_window(w):
            if w >= nwin or win_emitted[w]:
                return
            win_emitted[w] = True
            for s in window_steps(w):
                fill.append(s)

            def marker():
                win_done[w] = True
            fill.append(marker)

        def need_now(w):
            """Window w must be fully emitted before returning.

            Drains the FIFO only up to window w's own completion marker,
            leaving later windows / tail steps queued as jt-loop filler.
            """
            w = min(w, nwin - 1)
            push_window(w)
            while not win_done[w]:
                fill.popleft()()

        def finalize_qtile(cxs, gi0, icn, qt):
            """Normalize + transpose + out-project one completed qtile.

            Emitted immediately (gates reuse of the qtile's cx psum
            region): DVE reciprocal + raw-ctx evacuation, and Pool builds
            of diag(1/denom) = ident * recip. Queued as PE filler: the
            fused normalize-transpose matmuls (ctxU^T @ diag), the cT
            evacuation, and the out-projection.
            """
            cx = cxs[qt // 2]
            s = (qt % 2) * 130
            # evacuate + normalize: reciprocal of the psum denominator
            # columns, then one mult per head with the per-partition scalar
            # (hardware tensor_scalar has no divide)
            dn = dn_sb.tile([P, 2], F32, name="dn", tag="dn")
            nc.vector.reciprocal(dn, cx[:, s + 64:s + 130:65])
            cu = ctxn_sb.tile([P, 2, 64], BF16, name="cu", tag="cn")
            base = cx[:, s:s + 64]
            vsrc = bass.AP(tensor=base.tensor, offset=base.offset,
                           ap=[base.ap[0], [65, 2], [1, 64]])
            dnb = bass.AP(tensor=dn.tensor, offset=dn.offset,
                          ap=[dn.ap[0], [1, 2], [0, 64]])
            nc.vector.tensor_mul(cu, vsrc, dnb)
            s0 = gi0 + qt * P
            state = {}

            def t_step():
                tp = sm_ps.tile([P, P], BF16, name="tp", tag="sm")
                nc.tensor.transpose(tp, cu[:, :, :], ident_s)
                nc.vector.tensor_copy(cT[:, s0:s0 + P], tp)

            def o_step(nn):
                def step():
                    if nn == 0:
                        state["ob"] = ob_sb.tile([P, D], BF16,
                                                 name="ob", tag="ob")
                    ob = state["ob"]
                    op = sm_ps.tile([P, 512], F32, name="op", tag="sm")
                    nc.tensor.matmul(op, cT[:, s0:s0 + P],
                                     wo_s[:, nn * 512:(nn + 1) * 512],
                                     start=True, stop=True)
                    # split out-proj psum evacuations ACT/DVE by a modulo
                    # fraction: OBA8 of every 8 chunks go to ACT
                    ob_st["i"] += 1
                    if (ob_st["i"] * OBA8) % 8 < OBA8:
                        nc.scalar.copy(ob[:, nn * 512:(nn + 1) * 512], op)
                    else:
                        nc.vector.tensor_copy(
                            ob[:, nn * 512:(nn + 1) * 512], op)
                    if nn == D // 512 - 1:
                        if OUT_GP:
                            nc.gpsimd.dma_start(out=out[s0:s0 + P, :],
                                                in_=ob)
                        else:
                            nc.sync.dma_start(out=out[s0:s0 + P, :], in_=ob)
                return step

            return [t_step], [o_step(nn) for nn in range(D // 512)]

        # lag-N software pipeline for ctx matmuls, crossing chunk borders
        import os
        LAG = int(os.environ.get('K_LAG', '10'))
        RSV_MIN = int(os.environ.get('K_RSVMIN', '0'))
        TAILS_RSV = os.environ.get('K_TAILSRSV', '0') == '1'
        # fraction SCH_NUM/SCH_DEN of off-diagonal exp tiles computed on
        # DVE via the Schraudolph bit-trick instead of ACT
        SCH_NUM = int(os.environ.get('K_SCHNUM', '5'))
        SCH_DEN = int(os.environ.get('K_SCHDEN', '16'))
        SCH_PH = int(os.environ.get('K_SCHPH', '12'))
        sch_st = {"i": 0}
        ob_st = {"i": 0}
        # of every 8 out-proj evac chunks, this many go to ACT (rest DVE)
        OBA8 = int(os.environ.get('K_OBA8', '3'))
        OUT_GP = os.environ.get('K_OUTGP', '0') == '1'
        stash = deque()

        def emit_attn_chunk(b, icn):
            gi0 = b * S + icn * IC   # global query start
            njt = (icn + 1) * QT
            # bank tail work during PE-bound (small) chunks; spend it in
            # the ACT-bound (large) ones, and hold back a floor stock for
            # the filler-starved final chunks
            allow_rsv["on"] = njt >= RSV_MIN
            allow_rsv["keep"] = (0 if (b == B - 1 and icn >= NIC - 2)
                                 else int(os.environ.get('K_KEEP', '0')))
            ncx = (QT + 1) // 2
            cxs = [cx_ps.tile([P, 260], F32, name="cx", tag="cx")
                   for _ in range(ncx)]

            def make_ctx(jt, ex):
                def emit():
                    # One psum accumulation group per BANK (zero region):
                    # only the first matmul into each cx bank starts
                    # (lazily zeroing the whole bank); only the last one
                    # stops.
                    gjt = b * NJ + jt
                    for qt in range(QT):
                        if qt < jt - icn * QT:
                            continue  # fully masked block
                        cx = cxs[qt // 2]
                        s = (qt % 2) * 130
                        lastq = min(2 * (qt // 2) + 1, QT - 1)
                        for h in range(2):
                            nc.tensor.matmul(
                                cx[:, s + h * 65:s + (h + 1) * 65],
                                ex[:, h, qt * P:(qt + 1) * P],
                                vA[:, gjt, h * 65:(h + 1) * 65],
                                start=(jt == 0 and qt % 2 == 0 and h == 0),
                                stop=(jt == icn * QT + qt and qt == lastq
                                      and h == 1))
                    dq = jt - icn * QT
                    if dq >= 0 and (dq % 2 == 1 or dq == QT - 1):
                        # this cx bank's accumulation group just stopped:
                        # finalize both of its qtiles. All tail steps go
                        # to the paced reserve: the 1-pop-per-jt cadence
                        # spaces each PE step past its input evacuation.
                        all_t, all_o = [], []
                        for qt in range(2 * (dq // 2), dq + 1):
                            tsteps, osteps = finalize_qtile(
                                cxs, gi0, icn, qt)
                            all_t += tsteps
                            all_o += osteps
                        # both transposes first: doubles the spacing
                        # between each cT evacuation and its out-proj read
                        (reserve if TAILS_RSV else fill).extend(all_t)
                        reserve.extend(all_o)
                return emit

            for jt in range(njt):
                il0 = max(0, jt * P - icn * IC)
                gj0 = b * S + jt * P
                sc = sc_ps.tile([P, 2, IC], F32, name="sc", tag="sc")
                for h in range(2):
                    hp = h * 64
                    nc.tensor.matmul(
                        sc[:, h, il0:IC],
                        kT[hp:hp + 64, gj0:gj0 + P],
                        qT[hp:hp + 64, gi0 + il0:gi0 + IC],
                        start=True, stop=True)
                ex = exp_sb.tile([P, 2, IC], BF16, name="ex", tag="ex")
                diag = jt >= icn * QT
                sch_st["i"] += 1
                sch_now = ((sch_st["i"] + SCH_PH) * SCH_NUM) % SCH_DEN < SCH_NUM
                if not diag and SCH_NUM and sch_now:
                    # Schraudolph bit-trick exp on DVE (off-diagonal tiles
                    # only, ~1.8% rms weight error): fp16 bits of e^(s/8)
                    # are int16(round(1024/(8 ln2) * s + 15300.5)); one fused
                    # mult+add with int16 output aliasing the bf16 tile.
                    nc.vector.tensor_scalar(
                        ex[:, :, il0:IC].bitcast(mybir.dt.int16),
                        sc[:, :, il0:IC], 184.6649652337873, 15300.5,
                        ALU.mult, ALU.add)
                else:
                    nc.scalar.activation(ex[:, :, il0:IC], sc[:, :, il0:IC],
                                         AF.Exp, scale=0.125)
                if diag:  # diagonal tile: mask both heads in one Pool
                    # op (mask broadcast over the head dim; the ctx lag
                    # gives plenty of slack for Q7 latency)
                    d0 = jt * P - icn * IC
                    exd = ex[:, :, d0:d0 + P]
                    mb = bass.AP(tensor=mask_s.tensor, offset=mask_s.offset,
                                 ap=[mask_s.ap[0], [0, 2], [1, P]])
                    nc.gpsimd.tensor_mul(exd, exd, mb)
                pump(1)
                lag_now = 8 if (b == B - 1 and icn == NIC - 1) else LAG
                while len(stash) >= lag_now:
                    stash.popleft()()
                stash.append(make_ctx(jt, ex))
                pump(1)

        # -- main emission ---------------------------------------------
        push_window(0)
        need_now(0)
        push_window(1)
        import json as _json
        _ord = _json.loads(os.environ.get('K_ORD', 'null'))
        for b in range(B):
            icn_order = list(range(NIC))
            if _ord and b == B - 1:
                icn_order = [i for i in _ord if i < NIC]
                icn_order += [i for i in range(NIC) if i not in icn_order]
            for icn in icn_order:
                wlast = (b * S + (icn + 1) * IC - 1) // WN
                need_now(wlast)
                push_window(wlast + 1)
                push_window(wlast + 2)
                emit_attn_chunk(b, icn)
        while stash:
            stash.popleft()()
            pump(2)
        while fill:
            fill.popleft()()
        while reserve:
            reserve.popleft()()

    nc.compile()
    return nc


def _warrange(w, bf16):
    # [D, CW] -> [P, D//P, CW] contiguous (the SBUF layout, so the DMA is
    # a single contiguous copy instead of 256B strided pieces)
    D, CW_ = w.shape
    return np.ascontiguousarray(
        w.reshape(D // P, P, CW_).transpose(1, 0, 2)).astype(bf16)


def make_in_maps(x, Wq, Wk, Wv, Wo):
    bf16 = np.float16
    B, S, D = x.shape
    xT = np.ascontiguousarray(x.reshape(B * S, D).T).astype(bf16)
    mask = np.triu(np.ones((P, P), dtype=bf16))
    ident = np.eye(P, dtype=bf16)
    in_maps = []
    for c in range(N_CORES):
        cs = slice(c * CW, (c + 1) * CW)
        in_maps.append({
            "xT": xT,
            "wq": _warrange(Wq[:, cs], bf16),
            "wk": _warrange(Wk[:, cs], bf16),
            "wv": _warrange(Wv[:, cs], bf16),
            "wo": np.ascontiguousarray(Wo[cs, :]).astype(bf16),
            "mask": mask,
            "ident": ident,
        })
    return in_maps


_CACHED_NC = None


def kernel(x, Wq, Wk, Wv, Wo, bo, _trace=False):
    from concourse.bass_utils import run_bass_kernel_spmd
    global _CACHED_NC
    x = np.asarray(x, dtype=np.float32)
    B, S, D = x.shape
    if _CACHED_NC is None:
        _CACHED_NC = build_program(S=S, B=B, D=D)
    nc = _CACHED_NC
    in_maps = make_in_maps(x, np.asarray(Wq), np.asarray(Wk),
                           np.asarray(Wv), np.asarray(Wo))
    res = None
    for attempt in range(3):
        try:
            res = run_bass_kernel_spmd(nc, in_maps, list(range(N_CORES)),
                                       trace=_trace)
            break
        except Exception:
            if attempt == 2:
                raise
    out = np.zeros((B * S, D), dtype=np.float32)
    for c in range(N_CORES):
        out += res.results[c]["out"].astype(np.float32)
    out += np.asarray(bo, dtype=np.float32)[None, :]
    if _trace:
        kernel._last_result = res
    return out.reshape(B, S, D)



# revision 23
# speedup vs baseline: 1.0888x; 1.0254x over previous
"""Multi-head causal attention (B=2, S=4096, D=1024, H=16) on 8 TRN2 NeuronCores.

Sharding: head-parallel. Core c computes heads 2c, 2c+1 (128 of the 1024
projection columns) for both batches:
  - QKV column-parallel: each core gets Wq/Wk/Wv[:, c*128:(c+1)*128]
  - attention for its 2 heads over all tokens (causal)
  - out-proj row-parallel: partial_out = ctx_c @ Wo[c*128:(c+1)*128, :]
  - host sums the 8 partials and adds bo.

Engine layout (chosen against the TRN2 cost model):
  - PE: QKV projections, score matmuls (keys on psum partitions), ctx
    matmuls restructured with the exp'd scores as the STATIONARY operand
    and v (+ a ones column for the softmax denominator) as the 65-wide
    MOVING operand -- this halves ctx PE-rows vs. moving-exp form.
    Plus [q,dh]->[dh,q] ctx transposes and the out-projection.
  - ACT (scalar): the exp activations (the serial co-bottleneck).
  - DVE (vector): psum evacuations (q/k/v, ctx, most out-proj), softmax
    normalize (reciprocal + per-partition tensor_scalar mult), and an
    adaptive share of the exp tiles (larger in big chunks, where the
    per-jt window-load traffic is low) via the Schraudolph int16 bit
    trick on fp16 (~1.8% rms on those weights; storage is fp16 rather
    than bf16 everywhere, whose 8x lower quantization error pays for
    the bigger Schraudolph share: end-to-end rel err ~5e-3 vs 2e-2).
  - Pool (gpsimd): causal-mask multiplies (GPSIMD cannot touch PSUM).
Scheduling: one continuous jt stream across chunks with a lag-10 ctx
software pipeline, QKV windows emitted as self-contained filler steps
between attention ops, and deadline-free out-projection steps in a
reserve queue that feeds the ACT-bound late chunks.

Layouts on-chip (per core):
  qT, kT:  [128, T]  rows 0:64 head0, 64:128 head1 (transposed projections)
  vA:      [128, T/128, 130]  per key-tile [v_h0 | ones | v_h1 | ones]
  sc:      PSUM [128 keys, 2 heads, IC queries] -> exp on ACT -> ex SBUF
  cx:      PSUM [128 queries, 2*130] two query-tiles' ctx (+denominators
           in columns 64/129 via the ones columns)
  cT:      [128 = 2*64 dh, T]  normalized+transposed ctx
"""

from collections import deque
from contextlib import ExitStack

import numpy as np

import concourse.bass as bass
import concourse.tile as tile
from concourse import bacc, mybir

F32 = mybir.dt.float32
# 16-bit storage dtype: fp16 (10 mantissa bits) instead of bf16 -- ~8x
# lower quantization error at identical engine/matmul cost, freeing error
# budget for a much larger Schraudolph share of the exp work.
BF16 = mybir.dt.float16
P = 128
AF = mybir.ActivationFunctionType
ALU = mybir.AluOpType

N_CORES = 8
B_FULL, S_FULL, D_FULL, H_FULL = 2, 4096, 1024, 16
DH = 64
CW = 128  # projection columns per core (2 heads * 64)


def build_program(S=S_FULL, B=B_FULL, D=D_FULL):
    """Build the per-core Bass program (same program on all 8 cores)."""
    T = B * S
    KC = D // P            # contraction chunks for the projections
    IC = min(512, S)       # query-chunk width
    QT = IC // P           # 128-query tiles per chunk
    NJ = S // P            # key tiles per batch
    NIC = S // IC          # query chunks per batch
    WN = min(512, T)       # QKV token window
    nwin = T // WN

    nc = bacc.Bacc("TRN2", target_bir_lowering=False, debug=False,
                   num_devices=N_CORES)

    xT = nc.dram_tensor("xT", [D, T], BF16, kind="ExternalInput").ap()
    wq = nc.dram_tensor("wq", [P, KC, CW], BF16, kind="ExternalInput").ap()
    wk = nc.dram_tensor("wk", [P, KC, CW], BF16, kind="ExternalInput").ap()
    wv = nc.dram_tensor("wv", [P, KC, CW], BF16, kind="ExternalInput").ap()
    wo = nc.dram_tensor("wo", [CW, D], BF16, kind="ExternalInput").ap()
    mask = nc.dram_tensor("mask", [P, P], BF16, kind="ExternalInput").ap()
    ident = nc.dram_tensor("ident", [P, P], BF16, kind="ExternalInput").ap()
    out = nc.dram_tensor("out", [T, D], BF16, kind="ExternalOutput").ap()

    with tile.TileContext(nc) as tc, ExitStack() as ctx:
        singles = ctx.enter_context(tc.tile_pool(name="singles", bufs=1))
        qT = singles.tile([P, T], BF16, name="qT")
        kT = singles.tile([P, T], BF16, name="kT")
        vA = singles.tile([P, B * NJ, 130], BF16, name="vA")
        cT = singles.tile([P, T], BF16, name="cT")
        wq_s = singles.tile([P, KC, CW], BF16, name="wq_s")
        wk_s = singles.tile([P, KC, CW], BF16, name="wk_s")
        wv_s = singles.tile([P, KC, CW], BF16, name="wv_s")
        wo_s = singles.tile([CW, D], BF16, name="wo_s")
        mask_s = singles.tile([P, P], BF16, name="mask_s")
        ident_s = singles.tile([P, P], BF16, name="ident_s")

        # wq first: the very first matmuls need only wq + xw[0]; the rest
        # of the weight loads are interleaved into window 0's DMA step.
        # Split so the first matmul's kc-0/1 slice lands a transfer earlier
        # (singles tiles have subtile deps).
        nc.sync.dma_start(out=wq_s[:, 0:2, :], in_=wq[:, 0:2, :])
        nc.sync.dma_start(out=wq_s[:, 2:, :], in_=wq[:, 2:, :])
        nc.vector.memset(vA[:, :, 64:65], 1.0)
        nc.vector.memset(vA[:, :, 129:130], 1.0)

        xw_pool = ctx.enter_context(tc.tile_pool(name="xw_pool", bufs=4))
        import os as _os
        # PSUM budget (8 banks): sc SCB x2 + cx 2x1 + sm SMB x1 = 8
        SCB = int(_os.environ.get('K_SCB', '2'))
        SMB = int(_os.environ.get('K_SMB', '2'))
        sc_ps = ctx.enter_context(
            tc.tile_pool(name="sc_ps", bufs=SCB, space=bass.MemorySpace.PSUM))
        cx_ps = ctx.enter_context(
            tc.tile_pool(name="cx_ps", bufs=int(_os.environ.get('K_CXB', '2')),
                         space=bass.MemorySpace.PSUM))
        sm_ps = ctx.enter_context(
            tc.tile_pool(name="sm_ps", bufs=SMB, space=bass.MemorySpace.PSUM))
        exp_sb = ctx.enter_context(tc.tile_pool(
            name="exp_sb", bufs=int(_os.environ.get('K_EXB', '30'))))
        ctxn_sb = ctx.enter_context(tc.tile_pool(
            name="ctxn_sb", bufs=int(_os.environ.get("K_CNB", "12"))))
        dn_sb = ctx.enter_context(tc.tile_pool(name="dn_sb", bufs=4))
        scf_sb = ctx.enter_context(tc.tile_pool(
            name="scf_sb", bufs=int(_os.environ.get("K_SCFB", "3"))))
        ob_sb = ctx.enter_context(tc.tile_pool(
            name="ob_sb", bufs=int(_os.environ.get("K_OBB", "6"))))

        fill = deque()          # deferred emission steps (mostly PE filler)
        reserve = deque()       # deadline-free steps (out-projection):
        # drained only when fill is dry, feeding the filler-starved
        # ACT-bound late chunks
        win_emitted = [False] * nwin
        win_done = [False] * nwin

        allow_rsv = {"on": True, "keep": 0}

        def pump(n):
            for _ in range(n):
                if fill:
                    fill.popleft()()
                elif (reserve and allow_rsv["on"]
                        and len(reserve) > allow_rsv["keep"]):
                    reserve.popleft()()

        def window_steps(w):
            """Generate the emission steps for QKV window w."""
            xw = xw_pool.tile([P, KC, WN], BF16, name="xw", tag="xw")

            # DMAs issue eagerly at push time (no PE work): the loads are
            # in flight well before the compute steps get popped.
            if w == 0:
                # window 0 gates kernel start: per-kc loads spread over two
                # queues let the first matmul begin as soon as its slice lands
                for kc in range(KC):
                    eng = nc.scalar if kc % 2 == 1 else nc.sync
                    eng.dma_start(
                        out=xw[:, kc, :],
                        in_=xT[kc * P:(kc + 1) * P, w * WN:(w + 1) * WN])
            else:
                # two strided DMAs per window: [p, kc, wn] <- xT[kc*P+p,
                # w*WN+wn] (vs 8 per-kc: saves HWDGE passes + SP issues;
                # vs 1: the first proj part [kc 0:4] starts half a
                # transfer earlier)
                half = KC // 2
                for hh in range(2):
                    xsrc = bass.AP(tensor=xT.tensor,
                                   offset=w * WN + hh * half * P * T,
                                   ap=[[T, P], [P * T, half], [1, WN]])
                    nc.sync.dma_start(out=xw[:, hh * half:(hh + 1) * half, :],
                                      in_=xsrc)
            if w == 0:
                # deferred loads, ordered by first use
                nc.sync.dma_start(out=wk_s, in_=wk)
                nc.sync.dma_start(out=mask_s, in_=mask)
                nc.sync.dma_start(out=wv_s, in_=wv)
                nc.sync.dma_start(out=ident_s, in_=ident)
                nc.sync.dma_start(out=wo_s, in_=wo)

            state = {}
            # window 0 runs at kernel start with nothing to overlap:
            # per-kc steps let the first matmul start as soon as its own
            # wq/xw slices land. Later windows use ~850ns halves.
            nparts = KC if w == 0 else 2

            def proj_step(which, w_sb, dst, part):
                # fill holds only window steps so the parts pop adjacently
                # (no sm-pool interleave hazard)
                def step():
                    if part == 0:
                        state[which] = sm_ps.tile([P, WN], F32,
                                                  name=which, tag="sm")
                    ps = state[which]
                    for kc in range(part * KC // nparts,
                                    (part + 1) * KC // nparts):
                        nc.tensor.matmul(ps, w_sb[:, kc, :], xw[:, kc, :],
                                         start=(kc == 0),
                                         stop=(kc == KC - 1))
                    if part == nparts - 1:
                        nc.vector.tensor_copy(
                            dst[:, w * WN:(w + 1) * WN], ps)
                return step

            for part in range(nparts):
                yield proj_step("q_ps", wq_s, qT, part)
            for part in range(nparts):
                yield proj_step("k_ps", wk_s, kT, part)

            def v_step(st):
                def step():
                    jt = (w * WN) // P + st  # global token tile
                    vp = sm_ps.tile([P, CW], F32, name="vp", tag="sm")
                    for kc in range(KC):
                        nc.tensor.matmul(vp, xw[:, kc, st * P:(st + 1) * P],
                                         wv_s[:, kc, :],
                                         start=(kc == 0), stop=(kc == KC - 1))
                    # strided evac: [v_h0 | v_h1] -> vA cols {0:64, 65:129}
                    base = vA[:, jt, 0:64]
                    dst = bass.AP(tensor=base.tensor, offset=base.offset,
                                  ap=[base.ap[0], [65, 2], [1, 64]])
                    src = bass.AP(tensor=vp.tensor, offset=vp.offset,
                                  ap=[vp.ap[0], [64, 2], [1, 64]])
                    nc.vector.tensor_copy(dst, src)
                return step

            for st in range(WN // P):
                yield v_step(st)

        def push_window(w):
            if w >= nwin or win_emitted[w]:
                return
            win_emitted[w] = True
            for s in window_steps(w):
                fill.append(s)

            def marker():
                win_done[w] = True
            fill.append(marker)

        def need_now(w):
            """Window w must be fully emitted before returning.

            Drains the FIFO only up to window w's own completion marker,
            leaving later windows / tail steps queued as jt-loop filler.
            """
            w = min(w, nwin - 1)
            push_window(w)
            while not win_done[w]:
                fill.popleft()()

        def finalize_qtile(cxs, gi0, icn, qt):
            """Normalize + transpose + out-project one completed qtile.

            Emitted immediately (gates reuse of the qtile's cx psum
            region): DVE reciprocal + raw-ctx evacuation, and Pool builds
            of diag(1/denom) = ident * recip. Queued as PE filler: the
            fused normalize-transpose matmuls (ctxU^T @ diag), the cT
            evacuation, and the out-projection.
            """
            cx = cxs[qt // 2]
            s = (qt % 2) * 130
            # evacuate + normalize: reciprocal of the psum denominator
            # columns, then one mult per head with the per-partition scalar
            # (hardware tensor_scalar has no divide)
            dn = dn_sb.tile([P, 2], F32, name="dn", tag="dn")
            nc.vector.reciprocal(dn, cx[:, s + 64:s + 130:65])
            cu = ctxn_sb.tile([P, 2, 64], BF16, name="cu", tag="cn")
            base = cx[:, s:s + 64]
            vsrc = bass.AP(tensor=base.tensor, offset=base.offset,
                           ap=[base.ap[0], [65, 2], [1, 64]])
            dnb = bass.AP(tensor=dn.tensor, offset=dn.offset,
                          ap=[dn.ap[0], [1, 2], [0, 64]])
            nc.vector.tensor_mul(cu, vsrc, dnb)
            s0 = gi0 + qt * P
            state = {}

            def t_step():
                if T_DMA:
                    # xbar DMA transpose straight from cu SBUF to cT: takes
                    # the transpose off the in-order PE queue (no head-of-
                    # line stall on the normalize chain) and kills the DVE
                    # psum evacuation
                    nc.sync.dma_start_transpose(cT[:, s0:s0 + P], cu[:, :, :])
                else:
                    tp = sm_ps.tile([P, P], BF16, name="tp", tag="sm")
                    nc.tensor.transpose(tp, cu[:, :, :], ident_s)
                    nc.vector.tensor_copy(cT[:, s0:s0 + P], tp)

            def o_step(nn):
                def step():
                    if nn == 0:
                        state["ob"] = ob_sb.tile([P, D], BF16,
                                                 name="ob", tag="ob")
                    ob = state["ob"]
                    op = sm_ps.tile([P, 512], F32, name="op", tag="sm")
                    nc.tensor.matmul(op, cT[:, s0:s0 + P],
                                     wo_s[:, nn * 512:(nn + 1) * 512],
                                     start=True, stop=True)
                    # split out-proj psum evacuations ACT/DVE by a modulo
                    # fraction: OBA8 of every 8 chunks go to ACT
                    ob_st["i"] += 1
                    if (ob_st["i"] * OBA8) % 8 < OBA8:
                        nc.scalar.copy(ob[:, nn * 512:(nn + 1) * 512], op)
                    else:
                        nc.vector.tensor_copy(
                            ob[:, nn * 512:(nn + 1) * 512], op)
                    if s0 >= T - IC:
                        # kernel tail: DMA each half as soon as its evac
                        # lands instead of waiting for the full row block
                        nc.sync.dma_start(
                            out=out[s0:s0 + P, nn * 512:(nn + 1) * 512],
                            in_=ob[:, nn * 512:(nn + 1) * 512])
                    elif nn == D // 512 - 1:
                        if OUT_GP:
                            nc.gpsimd.dma_start(out=out[s0:s0 + P, :],
                                                in_=ob)
                        else:
                            nc.sync.dma_start(out=out[s0:s0 + P, :], in_=ob)
                return step

            return [t_step], [o_step(nn) for nn in range(D // 512)]

        # lag-N software pipeline for ctx matmuls, crossing chunk borders
        import os
        LAG = int(os.environ.get('K_LAG', '12'))
        RSV_MIN = int(os.environ.get('K_RSVMIN', '0'))
        TAILS_RSV = os.environ.get('K_TAILSRSV', '1') == '1'
        # fraction SCH_NUM/SCH_DEN of off-diagonal exp tiles computed on
        # DVE via the Schraudolph bit-trick instead of ACT
        SCH_NUM = int(os.environ.get('K_SCHNUM', '5'))
        SCH_DEN = int(os.environ.get('K_SCHDEN', '16'))
        SCH_PH = int(os.environ.get('K_SCHPH', '12'))
        # adaptive DVE-exp share: window-load DVE traffic is ~constant per
        # CHUNK, so per-jt DVE slack grows with chunk size while ACT's exp
        # load per jt stays flat -- shift exp toward DVE in big chunks.
        # eff_num(njt) = SCHA + SCHB*njt/32 sixteenths (overrides SCH_NUM
        # when K_SCHAD=1)
        SCH_AD = os.environ.get('K_SCHAD', '1') == '1'
        SCH_A = int(os.environ.get('K_SCHA', '3'))
        SCH_B = int(os.environ.get('K_SCHB', '4'))
        # of every 16 off-diagonal tiles, POOL16 go to the Pool lane:
        # DMA-stage sc (psum f32) into SBUF, Schraudolph on gpsimd
        POOL16 = 0  # dead: DMA cannot read PSUM (bass asserts src SBUF/DRAM)
        sch_st = {"i": 0}
        ob_st = {"i": 0}
        # of every 8 out-proj evac chunks, this many go to ACT (rest DVE)
        OBA8 = int(os.environ.get('K_OBA8', '3'))
        T_DMA = os.environ.get('K_TDMA', '0') == '1'
        PF = int(os.environ.get('K_PF', '2'))
        OUT_GP = os.environ.get('K_OUTGP', '0') == '1'
        stash = deque()

        def emit_attn_chunk(b, icn):
            gi0 = b * S + icn * IC   # global query start
            njt = (icn + 1) * QT
            # bank tail work during PE-bound (small) chunks; spend it in
            # the ACT-bound (large) ones, and hold back a floor stock for
            # the filler-starved final chunks
            allow_rsv["on"] = njt >= RSV_MIN
            allow_rsv["keep"] = (0 if (b == B - 1 and icn >= NIC - 2)
                                 else int(os.environ.get('K_KEEP', '16')))
            ncx = (QT + 1) // 2
            cxs = [cx_ps.tile([P, 260], F32, name="cx", tag="cx")
                   for _ in range(ncx)]

            def make_ctx(jt, ex):
                def emit():
                    # One psum accumulation group per BANK (zero region):
                    # only the first matmul into each cx bank starts
                    # (lazily zeroing the whole bank); only the last one
                    # stops.
                    gjt = b * NJ + jt
                    for qt in range(QT):
                        if qt < jt - icn * QT:
                            continue  # fully masked block
                        cx = cxs[qt // 2]
                        s = (qt % 2) * 130
                        lastq = min(2 * (qt // 2) + 1, QT - 1)
                        for h in range(2):
                            nc.tensor.matmul(
                                cx[:, s + h * 65:s + (h + 1) * 65],
                                ex[:, h, qt * P:(qt + 1) * P],
                                vA[:, gjt, h * 65:(h + 1) * 65],
                                start=(jt == 0 and qt % 2 == 0 and h == 0),
                                stop=(jt == icn * QT + qt and qt == lastq
                                      and h == 1))
                    dq = jt - icn * QT
                    if dq >= 0 and (dq % 2 == 1 or dq == QT - 1):
                        # this cx bank's accumulation group just stopped:
                        # finalize both of its qtiles. All tail steps go
                        # to the paced reserve: the 1-pop-per-jt cadence
                        # spaces each PE step past its input evacuation.
                        all_t, all_o = [], []
                        for qt in range(2 * (dq // 2), dq + 1):
                            tsteps, osteps = finalize_qtile(
                                cxs, gi0, icn, qt)
                            all_t += tsteps
                            all_o += osteps
                        # both transposes first: doubles the spacing
                        # between each cT evacuation and its out-proj read
                        (reserve if TAILS_RSV else fill).extend(all_t)
                        reserve.extend(all_o)
                return emit

            for jt in range(njt):
                il0 = max(0, jt * P - icn * IC)
                gj0 = b * S + jt * P
                sc = sc_ps.tile([P, 2, IC], F32, name="sc", tag="sc")
                for h in range(2):
                    hp = h * 64
                    nc.tensor.matmul(
                        sc[:, h, il0:IC],
                        kT[hp:hp + 64, gj0:gj0 + P],
                        qT[hp:hp + 64, gi0 + il0:gi0 + IC],
                        start=True, stop=True)
                ex = exp_sb.tile([P, 2, IC], BF16, name="ex", tag="ex")
                diag = jt >= icn * QT
                sch_st["i"] += 1
                pool_now = (POOL16
                            and (sch_st["i"] * POOL16) % 16 < POOL16)
                if SCH_AD:
                    num_eff = min(16, SCH_A + (SCH_B * njt) // 32)
                    sch_now = ((sch_st["i"] + SCH_PH) * num_eff) % 16 < num_eff
                else:
                    sch_now = ((sch_st["i"] + SCH_PH) * SCH_NUM) % SCH_DEN < SCH_NUM
                if not diag and pool_now:
                    # Pool exp lane: DMA evacuates the psum scores to SBUF
                    # (no engine time), gpsimd does the Schraudolph there.
                    scf = scf_sb.tile([P, 2, IC], F32, name="scf", tag="scf")
                    nc.sync.dma_start(out=scf[:, :, il0:IC],
                                      in_=sc[:, :, il0:IC])
                    nc.gpsimd.tensor_scalar(
                        ex[:, :, il0:IC].bitcast(mybir.dt.int16),
                        scf[:, :, il0:IC], 184.6649652337873, 15300.5,
                        ALU.mult, ALU.add)
                elif not diag and SCH_NUM and sch_now:
                    # Schraudolph bit-trick exp on DVE (off-diagonal tiles
                    # only, ~1.8% rms weight error): fp16 bits of e^(s/8)
                    # are int16(round(1024/(8 ln2) * s + 15300.5)); one fused
                    # mult+add with int16 output aliasing the bf16 tile.
                    nc.vector.tensor_scalar(
                        ex[:, :, il0:IC].bitcast(mybir.dt.int16),
                        sc[:, :, il0:IC], 184.6649652337873, 15300.5,
                        ALU.mult, ALU.add)
                else:
                    nc.scalar.activation(ex[:, :, il0:IC], sc[:, :, il0:IC],
                                         AF.Exp, scale=0.125)
                if diag:  # diagonal tile: mask both heads in one Pool
                    # op (mask broadcast over the head dim; the ctx lag
                    # gives plenty of slack for Q7 latency)
                    d0 = jt * P - icn * IC
                    exd = ex[:, :, d0:d0 + P]
                    mb = bass.AP(tensor=mask_s.tensor, offset=mask_s.offset,
                                 ap=[mask_s.ap[0], [0, 2], [1, P]])
                    nc.gpsimd.tensor_mul(exd, exd, mb)
                pump(1)
                lag_now = 8 if (b == B - 1 and icn == NIC - 1) else LAG
                while len(stash) >= lag_now:
                    stash.popleft()()
                stash.append(make_ctx(jt, ex))
                pump(1)

        # -- main emission ---------------------------------------------
        push_window(0)
        need_now(0)
        push_window(1)
        import json as _json
        _ord = _json.loads(os.environ.get('K_ORD', 'null'))
        for b in range(B):
            icn_order = list(range(NIC))
            if _ord and b == B - 1:
                icn_order = [i for i in _ord if i < NIC]
                icn_order += [i for i in range(NIC) if i not in icn_order]
            for icn in icn_order:
                wlast = (b * S + (icn + 1) * IC - 1) // WN
                need_now(wlast)
                for pf in range(1, PF + 1):
                    push_window(wlast + pf)
                emit_attn_chunk(b, icn)
        while stash:
            stash.popleft()()
            pump(2)
        while fill:
            fill.popleft()()
        while reserve:
            reserve.popleft()()

    nc.compile()
    return nc


def _warrange(w, bf16):
    # [D, CW] -> [P, D//P, CW] contiguous (the SBUF layout, so the DMA is
    # a single contiguous copy instead of 256B strided pieces)
    D, CW_ = w.shape
    return np.ascontiguousarray(
        w.reshape(D // P, P, CW_).transpose(1, 0, 2)).astype(bf16)


def make_in_maps(x, Wq, Wk, Wv, Wo):
    bf16 = np.float16
    B, S, D = x.shape
    xT = np.ascontiguousarray(x.reshape(B * S, D).T).astype(bf16)
    mask = np.triu(np.ones((P, P), dtype=bf16))
    ident = np.eye(P, dtype=bf16)
    in_maps = []
    for c in range(N_CORES):
        cs = slice(c * CW, (c + 1) * CW)
        in_maps.append({
            "xT": xT,
            "wq": _warrange(Wq[:, cs], bf16),
            "wk": _warrange(Wk[:, cs], bf16),
            "wv": _warrange(Wv[:, cs], bf16),
            "wo": np.ascontiguousarray(Wo[cs, :]).astype(bf16),
            "mask": mask,
            "ident": ident,
        })
    return in_maps


_CACHED_NC = None


def kernel(x, Wq, Wk, Wv, Wo, bo, _trace=False):
    from concourse.bass_utils import run_bass_kernel_spmd
    global _CACHED_NC
    x = np.asarray(x, dtype=np.float32)
    B, S, D = x.shape
    if _CACHED_NC is None:
        _CACHED_NC = build_program(S=S, B=B, D=D)
    nc = _CACHED_NC
    in_maps = make_in_maps(x, np.asarray(Wq), np.asarray(Wk),
                           np.asarray(Wv), np.asarray(Wo))
    res = None
    for attempt in range(3):
        try:
            res = run_bass_kernel_spmd(nc, in_maps, list(range(N_CORES)),
                                       trace=_trace)
            break
        except Exception:
            if attempt == 2:
                raise
    out = np.zeros((B * S, D), dtype=np.float32)
    for c in range(N_CORES):
        out += res.results[c]["out"].astype(np.float32)
    out += np.asarray(bo, dtype=np.float32)[None, :]
    if _trace:
        kernel._last_result = res
    return out.reshape(B, S, D)



# revision 24
# speedup vs baseline: 1.0946x; 1.0053x over previous
"""Multi-head causal attention (B=2, S=4096, D=1024, H=16) on 8 TRN2 NeuronCores.

Sharding: head-parallel. Core c computes heads 2c, 2c+1 (128 of the 1024
projection columns) for both batches:
  - QKV column-parallel: each core gets Wq/Wk/Wv[:, c*128:(c+1)*128]
  - attention for its 2 heads over all tokens (causal)
  - out-proj row-parallel: partial_out = ctx_c @ Wo[c*128:(c+1)*128, :]
  - host sums the 8 partials and adds bo.

Engine layout (chosen against the TRN2 cost model):
  - PE: QKV projections, score matmuls (keys on psum partitions), ctx
    matmuls restructured with the exp'd scores as the STATIONARY operand
    and v (+ a ones column for the softmax denominator) as the 65-wide
    MOVING operand -- this halves ctx PE-rows vs. moving-exp form.
    Plus [q,dh]->[dh,q] ctx transposes and the out-projection.
  - ACT (scalar): the exp activations (the serial co-bottleneck).
  - DVE (vector): psum evacuations (q/k/v, ctx, most out-proj), softmax
    normalize (reciprocal + per-partition tensor_scalar mult), and an
    adaptive share of the exp tiles (larger in big chunks, where the
    per-jt window-load traffic is low) via the Schraudolph int16 bit
    trick on fp16 (~1.8% rms on those weights; storage is fp16 rather
    than bf16 everywhere, whose 8x lower quantization error pays for
    the bigger Schraudolph share: end-to-end rel err ~5e-3 vs 2e-2).
  - Pool (gpsimd): causal-mask multiplies (GPSIMD cannot touch PSUM).
Scheduling: one continuous jt stream across chunks with a lag-10 ctx
software pipeline, QKV windows emitted as self-contained filler steps
between attention ops, and deadline-free out-projection steps in a
reserve queue that feeds the ACT-bound late chunks.

Layouts on-chip (per core):
  qT, kT:  [128, T]  rows 0:64 head0, 64:128 head1 (transposed projections)
  vA:      [128, T/128, 130]  per key-tile [v_h0 | ones | v_h1 | ones]
  sc:      PSUM [128 keys, 2 heads, IC queries] -> exp on ACT -> ex SBUF
  cx:      PSUM [128 queries, 2*130] two query-tiles' ctx (+denominators
           in columns 64/129 via the ones columns)
  cT:      [128 = 2*64 dh, T]  normalized+transposed ctx
"""

from collections import deque
from contextlib import ExitStack

import numpy as np

import concourse.bass as bass
import concourse.tile as tile
from concourse import bacc, mybir

F32 = mybir.dt.float32
# 16-bit storage dtype: fp16 (10 mantissa bits) instead of bf16 -- ~8x
# lower quantization error at identical engine/matmul cost, freeing error
# budget for a much larger Schraudolph share of the exp work.
BF16 = mybir.dt.float16
P = 128
AF = mybir.ActivationFunctionType
ALU = mybir.AluOpType

N_CORES = 8
B_FULL, S_FULL, D_FULL, H_FULL = 2, 4096, 1024, 16
DH = 64
CW = 128  # projection columns per core (2 heads * 64)


def build_program(S=S_FULL, B=B_FULL, D=D_FULL):
    """Build the per-core Bass program (same program on all 8 cores)."""
    T = B * S
    KC = D // P            # contraction chunks for the projections
    IC = min(512, S)       # query-chunk width
    QT = IC // P           # 128-query tiles per chunk
    NJ = S // P            # key tiles per batch
    NIC = S // IC          # query chunks per batch
    WN = min(512, T)       # QKV token window
    nwin = T // WN

    nc = bacc.Bacc("TRN2", target_bir_lowering=False, debug=False,
                   num_devices=N_CORES)

    xT = nc.dram_tensor("xT", [D, T], BF16, kind="ExternalInput").ap()
    wq = nc.dram_tensor("wq", [P, KC, CW], BF16, kind="ExternalInput").ap()
    wk = nc.dram_tensor("wk", [P, KC, CW], BF16, kind="ExternalInput").ap()
    wv = nc.dram_tensor("wv", [P, KC, CW], BF16, kind="ExternalInput").ap()
    wo = nc.dram_tensor("wo", [CW, D], BF16, kind="ExternalInput").ap()
    mask = nc.dram_tensor("mask", [P, P], BF16, kind="ExternalInput").ap()
    ident = nc.dram_tensor("ident", [P, P], BF16, kind="ExternalInput").ap()
    out = nc.dram_tensor("out", [T, D], BF16, kind="ExternalOutput").ap()

    with tile.TileContext(nc) as tc, ExitStack() as ctx:
        singles = ctx.enter_context(tc.tile_pool(name="singles", bufs=1))
        qT = singles.tile([P, T], BF16, name="qT")
        kT = singles.tile([P, T], BF16, name="kT")
        vA = singles.tile([P, B * NJ, 130], BF16, name="vA")
        cT = singles.tile([P, T], BF16, name="cT")
        wq_s = singles.tile([P, KC, CW], BF16, name="wq_s")
        wk_s = singles.tile([P, KC, CW], BF16, name="wk_s")
        wv_s = singles.tile([P, KC, CW], BF16, name="wv_s")
        wo_s = singles.tile([CW, D], BF16, name="wo_s")
        mask_s = singles.tile([P, P], BF16, name="mask_s")
        ident_s = singles.tile([P, P], BF16, name="ident_s")

        # wq first: the very first matmuls need only wq + xw[0]; the rest
        # of the weight loads are interleaved into window 0's DMA step.
        # Split so the first matmul's kc-0/1 slice lands a transfer earlier
        # (singles tiles have subtile deps).
        nc.sync.dma_start(out=wq_s[:, 0:2, :], in_=wq[:, 0:2, :])
        nc.sync.dma_start(out=wq_s[:, 2:, :], in_=wq[:, 2:, :])
        nc.vector.memset(vA[:, :, 64:65], 1.0)
        nc.vector.memset(vA[:, :, 129:130], 1.0)

        xw_pool = ctx.enter_context(tc.tile_pool(name="xw_pool", bufs=4))
        import os as _os
        # PSUM budget (8 banks): sc SCB x2 + cx 2x1 + sm SMB x1 = 8
        SCB = int(_os.environ.get('K_SCB', '2'))
        SMB = int(_os.environ.get('K_SMB', '2'))
        sc_ps = ctx.enter_context(
            tc.tile_pool(name="sc_ps", bufs=SCB, space=bass.MemorySpace.PSUM))
        cx_ps = ctx.enter_context(
            tc.tile_pool(name="cx_ps", bufs=int(_os.environ.get('K_CXB', '2')),
                         space=bass.MemorySpace.PSUM))
        sm_ps = ctx.enter_context(
            tc.tile_pool(name="sm_ps", bufs=SMB, space=bass.MemorySpace.PSUM))
        exp_sb = ctx.enter_context(tc.tile_pool(
            name="exp_sb", bufs=int(_os.environ.get('K_EXB', '30'))))
        ctxn_sb = ctx.enter_context(tc.tile_pool(
            name="ctxn_sb", bufs=int(_os.environ.get("K_CNB", "12"))))
        dn_sb = ctx.enter_context(tc.tile_pool(name="dn_sb", bufs=4))
        scf_sb = ctx.enter_context(tc.tile_pool(
            name="scf_sb", bufs=int(_os.environ.get("K_SCFB", "3"))))
        ob_sb = ctx.enter_context(tc.tile_pool(
            name="ob_sb", bufs=int(_os.environ.get("K_OBB", "6"))))

        fill = deque()          # deferred emission steps (mostly PE filler)
        reserve = deque()       # deadline-free steps (out-projection):
        # drained only when fill is dry, feeding the filler-starved
        # ACT-bound late chunks
        win_emitted = [False] * nwin
        win_done = [False] * nwin

        allow_rsv = {"on": True, "keep": 0}

        def pump(n):
            for _ in range(n):
                if fill:
                    fill.popleft()()
                elif (reserve and allow_rsv["on"]
                        and len(reserve) > allow_rsv["keep"]):
                    reserve.popleft()()

        def window_steps(w):
            """Generate the emission steps for QKV window w."""
            xw = xw_pool.tile([P, KC, WN], BF16, name="xw", tag="xw")

            # DMAs issue eagerly at push time (no PE work): the loads are
            # in flight well before the compute steps get popped.
            if w == 0:
                # window 0 gates kernel start: per-kc loads spread over two
                # queues let the first matmul begin as soon as its slice lands
                for kc in range(KC):
                    eng = nc.scalar if kc % 2 == 1 else nc.sync
                    eng.dma_start(
                        out=xw[:, kc, :],
                        in_=xT[kc * P:(kc + 1) * P, w * WN:(w + 1) * WN])
            else:
                # two strided DMAs per window: [p, kc, wn] <- xT[kc*P+p,
                # w*WN+wn] (vs 8 per-kc: saves HWDGE passes + SP issues;
                # vs 1: the first proj part [kc 0:4] starts half a
                # transfer earlier)
                half = KC // 2
                for hh in range(2):
                    xsrc = bass.AP(tensor=xT.tensor,
                                   offset=w * WN + hh * half * P * T,
                                   ap=[[T, P], [P * T, half], [1, WN]])
                    nc.sync.dma_start(out=xw[:, hh * half:(hh + 1) * half, :],
                                      in_=xsrc)
            if w == 0:
                # deferred loads, ordered by first use
                nc.sync.dma_start(out=wk_s, in_=wk)
                nc.sync.dma_start(out=mask_s, in_=mask)
                nc.sync.dma_start(out=wv_s, in_=wv)
                nc.sync.dma_start(out=ident_s, in_=ident)
                nc.sync.dma_start(out=wo_s, in_=wo)

            state = {}
            # window 0 runs at kernel start with nothing to overlap:
            # per-kc steps let the first matmul start as soon as its own
            # wq/xw slices land. Later windows use ~850ns halves.
            nparts = KC if w == 0 else 2

            def proj_step(which, w_sb, dst, part):
                # fill holds only window steps so the parts pop adjacently
                # (no sm-pool interleave hazard)
                def step():
                    if part == 0:
                        state[which] = sm_ps.tile([P, WN], F32,
                                                  name=which, tag="sm")
                    ps = state[which]
                    for kc in range(part * KC // nparts,
                                    (part + 1) * KC // nparts):
                        nc.tensor.matmul(ps, w_sb[:, kc, :], xw[:, kc, :],
                                         start=(kc == 0),
                                         stop=(kc == KC - 1))
                    if part == nparts - 1:
                        nc.vector.tensor_copy(
                            dst[:, w * WN:(w + 1) * WN], ps)
                return step

            for part in range(nparts):
                yield proj_step("q_ps", wq_s, qT, part)
            for part in range(nparts):
                yield proj_step("k_ps", wk_s, kT, part)

            def v_step(st):
                def step():
                    jt = (w * WN) // P + st  # global token tile
                    vp = sm_ps.tile([P, CW], F32, name="vp", tag="sm")
                    for kc in range(KC):
                        nc.tensor.matmul(vp, xw[:, kc, st * P:(st + 1) * P],
                                         wv_s[:, kc, :],
                                         start=(kc == 0), stop=(kc == KC - 1))
                    # strided evac: [v_h0 | v_h1] -> vA cols {0:64, 65:129}
                    base = vA[:, jt, 0:64]
                    dst = bass.AP(tensor=base.tensor, offset=base.offset,
                                  ap=[base.ap[0], [65, 2], [1, 64]])
                    src = bass.AP(tensor=vp.tensor, offset=vp.offset,
                                  ap=[vp.ap[0], [64, 2], [1, 64]])
                    nc.vector.tensor_copy(dst, src)
                return step

            for st in range(WN // P):
                yield v_step(st)

        def push_window(w):
            if w >= nwin or win_emitted[w]:
                return
            win_emitted[w] = True
            for s in window_steps(w):
                fill.append(s)

            def marker():
                win_done[w] = True
            fill.append(marker)

        def need_now(w):
            """Window w must be fully emitted before returning.

            Drains the FIFO only up to window w's own completion marker,
            leaving later windows / tail steps queued as jt-loop filler.
            """
            w = min(w, nwin - 1)
            push_window(w)
            while not win_done[w]:
                fill.popleft()()

        def finalize_qtile(cxs, gi0, icn, qt):
            """Normalize + transpose + out-project one completed qtile.

            Emitted immediately (gates reuse of the qtile's cx psum
            region): DVE reciprocal + raw-ctx evacuation, and Pool builds
            of diag(1/denom) = ident * recip. Queued as PE filler: the
            fused normalize-transpose matmuls (ctxU^T @ diag), the cT
            evacuation, and the out-projection.
            """
            cx = cxs[qt // 2]
            s = (qt % 2) * 130
            # evacuate + normalize: reciprocal of the psum denominator
            # columns, then one mult per head with the per-partition scalar
            # (hardware tensor_scalar has no divide)
            dn = dn_sb.tile([P, 2], F32, name="dn", tag="dn")
            nc.vector.reciprocal(dn, cx[:, s + 64:s + 130:65])
            cu = ctxn_sb.tile([P, 2, 64], BF16, name="cu", tag="cn")
            base = cx[:, s:s + 64]
            vsrc = bass.AP(tensor=base.tensor, offset=base.offset,
                           ap=[base.ap[0], [65, 2], [1, 64]])
            dnb = bass.AP(tensor=dn.tensor, offset=dn.offset,
                          ap=[dn.ap[0], [1, 2], [0, 64]])
            nc.vector.tensor_mul(cu, vsrc, dnb)
            s0 = gi0 + qt * P
            state = {}

            def t_step():
                if T_DMA:
                    # xbar DMA transpose straight from cu SBUF to cT: takes
                    # the transpose off the in-order PE queue (no head-of-
                    # line stall on the normalize chain) and kills the DVE
                    # psum evacuation
                    nc.sync.dma_start_transpose(cT[:, s0:s0 + P], cu[:, :, :])
                else:
                    tp = sm_ps.tile([P, P], BF16, name="tp", tag="sm")
                    nc.tensor.transpose(tp, cu[:, :, :], ident_s)
                    nc.vector.tensor_copy(cT[:, s0:s0 + P], tp)

            def o_step(nn):
                def step():
                    if nn == 0:
                        state["ob"] = ob_sb.tile([P, D], BF16,
                                                 name="ob", tag="ob")
                    ob = state["ob"]
                    op = sm_ps.tile([P, 512], F32, name="op", tag="sm")
                    nc.tensor.matmul(op, cT[:, s0:s0 + P],
                                     wo_s[:, nn * 512:(nn + 1) * 512],
                                     start=True, stop=True)
                    # split out-proj psum evacuations ACT/DVE by a modulo
                    # fraction: OBA8 of every 8 chunks go to ACT
                    ob_st["i"] += 1
                    if (ob_st["i"] * OBA8) % 8 < OBA8:
                        nc.scalar.copy(ob[:, nn * 512:(nn + 1) * 512], op)
                    else:
                        nc.vector.tensor_copy(
                            ob[:, nn * 512:(nn + 1) * 512], op)
                    if s0 >= T - IC:
                        # kernel tail: DMA each half as soon as its evac
                        # lands instead of waiting for the full row block
                        nc.sync.dma_start(
                            out=out[s0:s0 + P, nn * 512:(nn + 1) * 512],
                            in_=ob[:, nn * 512:(nn + 1) * 512])
                    elif nn == D // 512 - 1:
                        if OUT_GP:
                            nc.gpsimd.dma_start(out=out[s0:s0 + P, :],
                                                in_=ob)
                        else:
                            nc.sync.dma_start(out=out[s0:s0 + P, :], in_=ob)
                return step

            return [t_step], [o_step(nn) for nn in range(D // 512)]

        # lag-N software pipeline for ctx matmuls, crossing chunk borders
        import os
        LAG = int(os.environ.get('K_LAG', '12'))
        RSV_MIN = int(os.environ.get('K_RSVMIN', '0'))
        TAILS_RSV = os.environ.get('K_TAILSRSV', '1') == '1'
        # fraction SCH_NUM/SCH_DEN of off-diagonal exp tiles computed on
        # DVE via the Schraudolph bit-trick instead of ACT
        SCH_NUM = int(os.environ.get('K_SCHNUM', '5'))
        SCH_DEN = int(os.environ.get('K_SCHDEN', '16'))
        SCH_PH = int(os.environ.get('K_SCHPH', '8'))
        # adaptive DVE-exp share: window-load DVE traffic is ~constant per
        # CHUNK, so per-jt DVE slack grows with chunk size while ACT's exp
        # load per jt stays flat -- shift exp toward DVE in big chunks.
        # eff_num(njt) = SCHA + SCHB*njt/32 sixteenths (overrides SCH_NUM
        # when K_SCHAD=1)
        SCH_AD = os.environ.get('K_SCHAD', '1') == '1'
        SCH_A = int(os.environ.get('K_SCHA', '3'))
        SCH_B = int(os.environ.get('K_SCHB', '4'))
        # of every 16 off-diagonal tiles, POOL16 go to the Pool lane:
        # DMA-stage sc (psum f32) into SBUF, Schraudolph on gpsimd
        POOL16 = 0  # dead: DMA cannot read PSUM (bass asserts src SBUF/DRAM)
        sch_st = {"i": 0}
        ob_st = {"i": 0}
        # of every 8 out-proj evac chunks, this many go to ACT (rest DVE)
        OBA8 = int(os.environ.get('K_OBA8', '3'))
        T_DMA = os.environ.get('K_TDMA', '0') == '1'
        PF = int(os.environ.get('K_PF', '2'))
        OUT_GP = os.environ.get('K_OUTGP', '0') == '1'
        stash = deque()

        def emit_attn_chunk(b, icn):
            gi0 = b * S + icn * IC   # global query start
            njt = (icn + 1) * QT
            # bank tail work during PE-bound (small) chunks; spend it in
            # the ACT-bound (large) ones, and hold back a floor stock for
            # the filler-starved final chunks
            allow_rsv["on"] = njt >= RSV_MIN
            allow_rsv["keep"] = (0 if (b == B - 1 and icn >= NIC - 2)
                                 else int(os.environ.get('K_KEEP', '16')))
            ncx = (QT + 1) // 2
            cxs = [cx_ps.tile([P, 260], F32, name="cx", tag="cx")
                   for _ in range(ncx)]

            def make_ctx(jt, ex):
                def emit():
                    # One psum accumulation group per BANK (zero region):
                    # only the first matmul into each cx bank starts
                    # (lazily zeroing the whole bank); only the last one
                    # stops.
                    gjt = b * NJ + jt
                    for qt in range(QT):
                        if qt < jt - icn * QT:
                            continue  # fully masked block
                        cx = cxs[qt // 2]
                        s = (qt % 2) * 130
                        lastq = min(2 * (qt // 2) + 1, QT - 1)
                        for h in range(2):
                            nc.tensor.matmul(
                                cx[:, s + h * 65:s + (h + 1) * 65],
                                ex[:, h, qt * P:(qt + 1) * P],
                                vA[:, gjt, h * 65:(h + 1) * 65],
                                start=(jt == 0 and qt % 2 == 0 and h == 0),
                                stop=(jt == icn * QT + qt and qt == lastq
                                      and h == 1))
                    dq = jt - icn * QT
                    if dq >= 0 and (dq % 2 == 1 or dq == QT - 1):
                        # this cx bank's accumulation group just stopped:
                        # finalize both of its qtiles. All tail steps go
                        # to the paced reserve: the 1-pop-per-jt cadence
                        # spaces each PE step past its input evacuation.
                        all_t, all_o = [], []
                        for qt in range(2 * (dq // 2), dq + 1):
                            tsteps, osteps = finalize_qtile(
                                cxs, gi0, icn, qt)
                            all_t += tsteps
                            all_o += osteps
                        # both transposes first: doubles the spacing
                        # between each cT evacuation and its out-proj read
                        (reserve if TAILS_RSV else fill).extend(all_t)
                        reserve.extend(all_o)
                return emit

            for jt in range(njt):
                il0 = max(0, jt * P - icn * IC)
                gj0 = b * S + jt * P
                sc = sc_ps.tile([P, 2, IC], F32, name="sc", tag="sc")
                for h in range(2):
                    hp = h * 64
                    nc.tensor.matmul(
                        sc[:, h, il0:IC],
                        kT[hp:hp + 64, gj0:gj0 + P],
                        qT[hp:hp + 64, gi0 + il0:gi0 + IC],
                        start=True, stop=True)
                ex = exp_sb.tile([P, 2, IC], BF16, name="ex", tag="ex")
                diag = jt >= icn * QT
                sch_st["i"] += 1
                pool_now = (POOL16
                            and (sch_st["i"] * POOL16) % 16 < POOL16)
                if SCH_AD:
                    num_eff = min(16, SCH_A + (SCH_B * njt) // 32)
                    sch_now = ((sch_st["i"] + SCH_PH) * num_eff) % 16 < num_eff
                else:
                    sch_now = ((sch_st["i"] + SCH_PH) * SCH_NUM) % SCH_DEN < SCH_NUM
                if not diag and pool_now:
                    # Pool exp lane: DMA evacuates the psum scores to SBUF
                    # (no engine time), gpsimd does the Schraudolph there.
                    scf = scf_sb.tile([P, 2, IC], F32, name="scf", tag="scf")
                    nc.sync.dma_start(out=scf[:, :, il0:IC],
                                      in_=sc[:, :, il0:IC])
                    nc.gpsimd.tensor_scalar(
                        ex[:, :, il0:IC].bitcast(mybir.dt.int16),
                        scf[:, :, il0:IC], 184.6649652337873, 15300.5,
                        ALU.mult, ALU.add)
                elif not diag and SCH_NUM and sch_now:
                    # Schraudolph bit-trick exp on DVE (off-diagonal tiles
                    # only, ~1.8% rms weight error): fp16 bits of e^(s/8)
                    # are int16(round(1024/(8 ln2) * s + 15300.5)); one fused
                    # mult+add with int16 output aliasing the bf16 tile.
                    nc.vector.tensor_scalar(
                        ex[:, :, il0:IC].bitcast(mybir.dt.int16),
                        sc[:, :, il0:IC], 184.6649652337873, 15300.5,
                        ALU.mult, ALU.add)
                else:
                    nc.scalar.activation(ex[:, :, il0:IC], sc[:, :, il0:IC],
                                         AF.Exp, scale=0.125)
                if diag:  # diagonal tile: mask both heads in one Pool
                    # op (mask broadcast over the head dim; the ctx lag
                    # gives plenty of slack for Q7 latency)
                    d0 = jt * P - icn * IC
                    exd = ex[:, :, d0:d0 + P]
                    mb = bass.AP(tensor=mask_s.tensor, offset=mask_s.offset,
                                 ap=[mask_s.ap[0], [0, 2], [1, P]])
                    nc.gpsimd.tensor_mul(exd, exd, mb)
                pump(1)
                lag_now = 8 if (b == B - 1 and icn == NIC - 1) else LAG
                while len(stash) >= lag_now:
                    stash.popleft()()
                stash.append(make_ctx(jt, ex))
                pump(1)

        # -- main emission ---------------------------------------------
        push_window(0)
        need_now(0)
        push_window(1)
        import json as _json
        _ord = _json.loads(os.environ.get('K_ORD', 'null'))
        for b in range(B):
            icn_order = list(range(NIC))
            if _ord and b == B - 1:
                icn_order = [i for i in _ord if i < NIC]
                icn_order += [i for i in range(NIC) if i not in icn_order]
            for icn in icn_order:
                wlast = (b * S + (icn + 1) * IC - 1) // WN
                need_now(wlast)
                for pf in range(1, PF + 1):
                    push_window(wlast + pf)
                emit_attn_chunk(b, icn)
        while stash:
            stash.popleft()()
            pump(2)
        while fill:
            fill.popleft()()
        while reserve:
            reserve.popleft()()

    nc.compile()
    return nc


def _warrange(w, bf16):
    # [D, CW] -> [P, D//P, CW] contiguous (the SBUF layout, so the DMA is
    # a single contiguous copy instead of 256B strided pieces)
    D, CW_ = w.shape
    return np.ascontiguousarray(
        w.reshape(D // P, P, CW_).transpose(1, 0, 2)).astype(bf16)


def make_in_maps(x, Wq, Wk, Wv, Wo):
    bf16 = np.float16
    B, S, D = x.shape
    xT = np.ascontiguousarray(x.reshape(B * S, D).T).astype(bf16)
    mask = np.triu(np.ones((P, P), dtype=bf16))
    ident = np.eye(P, dtype=bf16)
    in_maps = []
    for c in range(N_CORES):
        cs = slice(c * CW, (c + 1) * CW)
        in_maps.append({
            "xT": xT,
            "wq": _warrange(Wq[:, cs], bf16),
            "wk": _warrange(Wk[:, cs], bf16),
            "wv": _warrange(Wv[:, cs], bf16),
            "wo": np.ascontiguousarray(Wo[cs, :]).astype(bf16),
            "mask": mask,
            "ident": ident,
        })
    return in_maps


_CACHED_NC = None


def kernel(x, Wq, Wk, Wv, Wo, bo, _trace=False):
    from concourse.bass_utils import run_bass_kernel_spmd
    global _CACHED_NC
    x = np.asarray(x, dtype=np.float32)
    B, S, D = x.shape
    if _CACHED_NC is None:
        _CACHED_NC = build_program(S=S, B=B, D=D)
    nc = _CACHED_NC
    in_maps = make_in_maps(x, np.asarray(Wq), np.asarray(Wk),
                           np.asarray(Wv), np.asarray(Wo))
    res = None
    for attempt in range(3):
        try:
            res = run_bass_kernel_spmd(nc, in_maps, list(range(N_CORES)),
                                       trace=_trace)
            break
        except Exception:
            if attempt == 2:
                raise
    out = np.zeros((B * S, D), dtype=np.float32)
    for c in range(N_CORES):
        out += res.results[c]["out"].astype(np.float32)
    out += np.asarray(bo, dtype=np.float32)[None, :]
    if _trace:
        kernel._last_result = res
    return out.reshape(B, S, D)

